# revision 40
# baseline (speedup 1.0000x reference)
"""BinaryLinear Trainium2 kernel: out = sign(x) @ sign(W).T

x: (4, 4096, 1024) f32, W: (1024, 1024) f32 -> out (4, 4096, 1024) f32.

Strategy (8 NeuronCores, data-parallel over flattened batch*seq rows):
  - Each core gets a [2048, 1024] row-shard of x and the full W.
  - Per 256-row fused tile (2 m-tiles):
      DMA x [128p, 2a, 1024i] f32 -> ACT Sign (fp32 -> fp8e4, +-1/0 exact)
      -> xbar DMA transpose of the fp8 bytes viewed as u16 pairs, giving
         [128p, 2a, 4c, 128m, 2b] with contraction index i = 256c + 2p + b
      -> per m-subtile 8 fp8 DoubleRow matmuls accumulate [128m, 1024o] in PSUM
      -> DVE copy PSUM -> SBUF as fp16 -> DMA out.
  - fp16 output: the sums are exact integers <= 1024 < 2048, exactly
    representable in fp16 - halves output traffic; host upcasts to f32.
  - Queue discipline (the scheduling insight this kernel is built around):
      * num_swdge_queues=1 so the Pool SWDGE ring drains strictly FIFO; all 8
        x-loads are front-loaded, so load 0 completes in ~3us and the
        sign/transpose/matmul pipeline starts immediately.
      * W (1MB, host-packed fp8) is enqueued after the first two x-loads:
        ready just before the first matmul needs it.
      * Output DMAs sit at the end of the Pool program, so their semaphore
        waits (on DVE copies) can never head-of-line-block a load enqueue;
        the last tile's store is split per-128-rows to trim the tail.
      * No flood-pausing gates: every variant that paused loads to protect
        the xbar transposes (whose 256B descriptors get ~1/17 of the DMA
        engines' round-robin against 4KB load packets) measured worse -
        each gate idles the engines ~5us, more than the crawl it prevents.
  - _fix_false_dma_coupling replaces Tile's conservative lane-aliased waits
    with exact producer-based waits; _legalize_dma_waits enforces walrus'
    per-instruction wait caps.

All arithmetic is exact; rel err vs the reference is 0.
"""

import numpy as np

P = 128
K = 1024  # in_features
N = 1024  # out_features
N_CORES = 8
M_TOTAL = 4 * 4096
M_PER_CORE = M_TOTAL // N_CORES
FUSE = 2


def build_binary_linear(tc, out, x, w):
    """Emit the per-core Tile kernel.

    out: DRAM [M, 1024] f16, x: DRAM [M, 1024] f32, w: DRAM [128, 8192] fp8.
    """
    import concourse.mybir as mybir

    nc = tc.nc
    f32 = mybir.dt.float32
    f16 = mybir.dt.float16
    fp8 = mybir.dt.float8e4
    u16 = mybir.dt.uint16
    Sign = mybir.ActivationFunctionType.Sign
    DR = mybir.MatmulPerfMode.DoubleRow

    M = x.shape[0]
    assert M % (FUSE * P) == 0 and x.shape[1] == K and w.shape == (P, 8 * N)
    n_fused = M // (FUSE * P)

    with (
        tc.tile_pool(name="wsb", bufs=1) as wpool,
        tc.tile_pool(name="xin", bufs=n_fused) as xin_pool,
        tc.tile_pool(name="x8p", bufs=n_fused) as x8_pool,
        tc.tile_pool(name="xt", bufs=8) as xt_pool,
        tc.tile_pool(name="osb", bufs=4) as out_pool,
        tc.tile_pool(name="ps", bufs=4, space="PSUM") as psum_pool,
    ):
        # ---- all x loads front-loaded on the FIFO SWDGE ring; W after the
        # first two so it lands just before the first matmul needs it ----
        xfs = []
        wT = None
        w5 = None
        for t in range(n_fused):
            r0 = t * FUSE * P
            xf = xin_pool.tile([P, FUSE, K], f32, tag="xf", name=f"xf_t{t}")
            nc.gpsimd.dma_start(
                out=xf, in_=x[r0 : r0 + FUSE * P].rearrange("(a p) i -> p a i", p=P)
            )
            xfs.append(xf)
            if t == 1:
                # W: host-packed fp8 [128, 8*1024]; wT[p, (2c+b)*1024 + o]
                # = sign(W)[o, i] with i = 256c + 2p + b. One 1MB DMA.
                wT = wpool.tile([P, 8 * N], fp8)
                nc.gpsimd.dma_start(out=wT, in_=w)
                # view for matmul rhs slices: [p][jj][d][b][o]; c = 2*jj + d
                w5 = wT.rearrange("p (jj d b o) -> p jj d b o", jj=2, d=2, b=2)

        osbs = []
        for t in range(n_fused):
            x8 = x8_pool.tile([P, FUSE, K], fp8, tag="x8", name=f"x8_t{t}")
            nc.scalar.activation(out=x8, in_=xfs[t], func=Sign)

            xt = xt_pool.tile([P, FUSE * 4 * P * 2], fp8, tag="xt")
            nc.sync.dma_start_transpose(
                out=xt.bitcast(u16).rearrange("p (cc m) -> p cc m", cc=4 * FUSE),
                in_=x8.bitcast(u16).rearrange("p a u -> p (a u)"),
            )
            # x5[p, a, c, m, b] = sign(x[r0 + 128a + m, 256c + 2p + b])
            x5 = xt.rearrange("p (a c m b) -> p a c m b", a=FUSE, c=4, b=2)

            osb = out_pool.tile([P, FUSE, N], f16, tag="osb")
            osbs.append(osb)
            for a in range(FUSE):
                ps = [
                    psum_pool.tile([P, 512], f32, tag="ps0", name="ps0"),
                    psum_pool.tile([P, 512], f32, tag="ps1", name="ps1"),
                ]
                for idx, (j, b) in enumerate(((0, 0), (0, 1), (1, 0), (1, 1))):
                    lhsT = x5[:, a, 2 * j : 2 * j + 2, :, b]  # [p][c:2][m:128]
                    for h in range(2):
                        nc.tensor.matmul(
                            ps[h],
                            lhsT=lhsT,
                            rhs=w5[:, j, :, b, h * 512 : (h + 1) * 512],
                            start=(idx == 0),
                            stop=(idx == 3),
                            perf_mode=DR,
                        )
                for h in range(2):
                    nc.vector.tensor_copy(
                        out=osb[:, a, h * 512 : (h + 1) * 512], in_=ps[h]
                    )
        # all out-DMAs at the end of the Pool program: their waits (on DVE
        # copies) cannot head-of-line-block any load enqueue; the last tile's
        # out is split per-128-rows so the final store only waits on its own
        # half's copies, trimming the tail
        for t in range(n_fused):
            r0 = t * FUSE * P
            if t < n_fused - 1:
                nc.gpsimd.dma_start(
                    out=out[r0 : r0 + FUSE * P].rearrange("(a p) i -> p a i", p=P),
                    in_=osbs[t],
                )
            else:
                for a in range(FUSE):
                    nc.gpsimd.dma_start(
                        out=out[r0 + a * P : r0 + (a + 1) * P], in_=osbs[t][:, a, :]
                    )


def _fix_false_dma_coupling(nc, n_xt_bufs, n_o_bufs):
    """Replace Tile's over-conservative / lane-aliased DMA waits with exact
    producer-based waits computed from the scheduled stream. Tile's sem pass
    expresses old slot-WAR deps via "dominating" recent (sometimes *future*)
    DMA-lane events, which couples the pipeline into lock-step. The true
    dependency structure per fused tile t (all loads/signs have dedicated
    buffers, so no WAR there):

      xf_load[t]     <- (free-running: all loads enqueue immediately on
                                        the FIFO SWDGE ring; any pacing gate
                                        measured strictly worse - it idles
                                        the DMA engines for longer than the
                                        transpose starvation it prevents)
      x8_sign[t]     <- xf_load[t]                      (RAW)
      xt_xpose[t]    <- x8_sign[t], mm_last[t - BT]     (RAW, xt-slot WAR)
      pe_first[t,0]  <- xt_xpose[t], wT (t==0),
                        copy[t-1, 0, 1]                 (RAW, psum WAR)
      pe_first[t,1]  <- copy[t-1, 1, 1]                 (psum WAR)
      copy[t,a,h]    <- mm_stop[t,a,h], out[t - BO]     (RAW, osb-slot WAR)
      out[t]         <- copy[t, 1, 1]                   (RAW)

    Waits are emitted as (producer's update-sem >= cumulative value after it).
    PE-queue instructions other than the first per a-group need no waits
    (FIFO). Soundness is validated by CoreSim in the dev harness.
    """
    import concourse.mybir as mybir

    insts = []
    for f in nc.m.functions:
        for bb in f.blocks:
            insts.extend(bb.instructions)

    cum = {}
    upd_after = {}  # inst name -> (sem_name, sem_id, cum_value_after)
    lane_order = {}  # inst name -> SyncWait enforcing same-lane completion order
    seqs = {k: [] for k in ("wT", "xf", "x8", "xt", "osb", "out")}
    pe_seq = []  # (LDW | MM) in PE-queue order
    for ins in insts:
        tn = type(ins).__name__
        if tn in ("InstLdweights", "InstMatmult"):
            pe_seq.append(ins)
        si = getattr(ins, "sync_info", None)
        if si is None:
            continue
        for u in si.on_update or []:
            prev = cum.get(u.ant_name, 0)
            if prev > 0 and (
                u.ant_name.startswith("DMAHW") or u.ant_name.startswith("DMASW")
            ):
                # DMA completions on one lane sem are not ordered by the HW;
                # the n-th updater must wait for the (n-1)-th's value or a
                # consumer's >= wait could be satisfied by the wrong DMA.
                lane_order[ins.name] = mybir.SyncWait(
                    sync_type="semaphore",
                    id=u.id,
                    ant_name=u.ant_name,
                    wait_mode="sem-ge-imm",
                    wait_value=prev,
                )
            cum[u.ant_name] = prev + u.update_value
            upd_after[ins.name] = (u.ant_name, u.id, cum[u.ant_name])
        if tn in ("InstLdweights", "InstMatmult"):
            continue
        memref = str(getattr(ins.outs[0], "memref", "")) if ins.outs else ""
        pref = memref.split("_")[0] if memref else ""
        for want_pref, want_tn in (
            ("wT", "InstDMACopy"),
            ("xf", "InstDMACopy"),
            ("x8", "InstActivation"),
            ("xt", "InstDmaTransposeAnt"),
            ("osb", "InstTensorCopy"),
            ("out", "InstDMACopy"),
        ):
            if tn == want_tn and pref == want_pref:
                seqs[want_pref].append(ins)
                break

    n_f = len(seqs["xt"])
    assert len(seqs["xf"]) == n_f and len(seqs["out"]) == n_f + 1, seqs
    assert len(seqs["osb"]) == 4 * n_f and len(pe_seq) == 32 * n_f, (
        len(seqs["osb"]),
        len(pe_seq),
    )

    def wait_on(producer_ins):
        sem_name, sem_id, v = upd_after[producer_ins.name]
        return mybir.SyncWait(
            sync_type="semaphore",
            id=sem_id,
            ant_name=sem_name,
            wait_mode="sem-ge-imm",
            wait_value=v,
        )

    def set_waits(ins, producers):
        si = getattr(ins, "sync_info", None)
        waits = [wait_on(p) for p in producers if p is not None]
        lo = lane_order.get(ins.name)
        if lo is not None:
            waits.append(lo)
        if si is None and not waits:
            return
        ins.sync_info = mybir.SyncInfo(
            on_wait=waits,
            on_update=list(si.on_update or []) if si is not None else [],
        )

    def mm(t, a, k):  # k-th matmul (0..7) of subtile (t, a)
        return pe_seq[32 * t + 16 * a + 2 * k + 1]

    def copy(t, a, h):
        return seqs["osb"][4 * t + 2 * a + h]

    for t, ins in enumerate(seqs["xf"]):
        # free-running: all loads enqueue immediately on the FIFO SWDGE
        # ring. Every flood-pausing gate variant measured strictly worse
        # (~5us of engine idle + resume latency per gate), even the one that
        # pulled the first matmul 12us earlier - the matmul END is bounded
        # by when the crawling transposes drain, not by the matmul start.
        set_waits(ins, [])
    set_waits(seqs["wT"][0], [])
    for t, ins in enumerate(seqs["x8"]):
        set_waits(ins, [seqs["xf"][t]])
    for t, ins in enumerate(seqs["xt"]):
        war = mm(t - n_xt_bufs, 1, 7) if t >= n_xt_bufs else None
        set_waits(ins, [seqs["x8"][t], war])
    for t in range(n_f):
        for a in range(2):
            prods = []
            if a == 0:
                prods.append(seqs["xt"][t])
                if t == 0:
                    prods.append(seqs["wT"][0])
            if t >= 2:  # psum bufs=4 -> one fused tile (2 a-groups) in flight
                prods.append(copy(t - 2, a, 1))
            set_waits(pe_seq[32 * t + 16 * a], prods)
            for k in range(1, 16):
                set_waits(pe_seq[32 * t + 16 * a + k], [])
    for t in range(n_f):
        for a in range(2):
            for h in range(2):
                war = seqs["out"][t - n_o_bufs] if t >= n_o_bufs else None
                set_waits(copy(t, a, h), [mm(t, a, 6 + h), war])
    for t, ins in enumerate(seqs["out"][: n_f - 1]):
        set_waits(ins, [copy(t, 1, 1)])
    set_waits(seqs["out"][n_f - 1], [copy(n_f - 1, 0, 1)])
    set_waits(seqs["out"][n_f], [copy(n_f - 1, 1, 1)])
    return {k: len(v) for k, v in seqs.items()}


def _legalize_dma_waits(nc):
    """Walrus caps in-struct sem waits: DMA_DIRECT2D_XPOSE takes 1, DMACopy 2.

    Tile's sem assignment is not transitively minimal and can emit 2-4 waits
    on DMA instructions. Hoist the excess into InstEventSemaphore wait-only
    instructions inserted just before the DMA on its triggering queue. This
    is sound: the queue executes the hoisted wait strictly before pushing the
    DMA descriptor, so the dependency is enforced (more conservatively) at
    trigger time instead of ring-pop time.
    """
    import concourse.mybir as mybir

    limits = {
        "InstDmaTransposeAnt": 1,
        "InstDMACopy": 1,
        "InstTensorCopy": 1,
        "InstActivation": 1,
        "InstMatmult": 1,
        "InstLdweights": 1,
        "InstMemset": 1,
        "InstTensorTensor": 1,
        "InstDrain": 1,
    }
    n_hoisted = 0
    for f in nc.m.functions:
        for bb in f.blocks:
            new_list = []
            for ins in bb.instructions:
                lim = limits.get(type(ins).__name__)
                si = getattr(ins, "sync_info", None)
                waits = list(si.on_wait) if si is not None and si.on_wait else []
                if lim is not None and len(waits) > lim:
                    # keep data-producer (engine-sem) waits in-struct first,
                    # then the freshest DMA-lane waits; hoist the rest
                    def keep_rank(w):
                        is_lane = w.ant_name.startswith(
                            "DMAHW"
                        ) or w.ant_name.startswith("DMASW")
                        return (1 if is_lane else 0, -w.wait_value)

                    waits_sorted = sorted(waits, key=keep_rank)
                    keep, hoist = waits_sorted[:lim], waits_sorted[lim:]
                    for ci in range(0, len(hoist), 2):
                        chunk = hoist[ci : ci + 2]
                        ev = mybir.InstEventSemaphore(
                            name=f"{ins.name}-prewait{ci // 2}",
                            engine=ins.engine,
                            ins=[],
                            outs=[],
                            sync_info=mybir.SyncInfo(on_wait=chunk, on_update=[]),
                        )
                        nc.inst_map[ev.name] = ev
                        new_list.append(ev)
                        n_hoisted += len(chunk)
                    ins.sync_info = mybir.SyncInfo(
                        on_wait=keep, on_update=list(si.on_update or [])
                    )
                new_list.append(ins)
            bb.instructions[:] = new_list
    return n_hoisted


def _build_nc(m_per_core):
    import concourse.bass as bass
    import concourse.mybir as mybir
    from concourse import tile

    nc = bass.Bass("TRN2", target_bir_lowering=False, num_swdge_queues=1)
    x_d = nc.dram_tensor("x", [m_per_core, K], mybir.dt.float32, kind="ExternalInput")
    w_d = nc.dram_tensor("W", [P, 8 * N], mybir.dt.float8e4, kind="ExternalInput")
    out_d = nc.dram_tensor(
        "out", [m_per_core, N], mybir.dt.float16, kind="ExternalOutput"
    )
    with tile.TileContext(nc) as tc:
        build_binary_linear(tc, out_d.ap(), x_d.ap(), w_d.ap())
    _fix_false_dma_coupling(nc, n_xt_bufs=8, n_o_bufs=4)
    _legalize_dma_waits(nc)
    return nc


_cached = {}


def _get_nc(m_per_core):
    if m_per_core not in _cached:
        _cached[m_per_core] = _build_nc(m_per_core)
    return _cached[m_per_core]


def kernel(x, W, _trace=False):
    from concourse import bass_utils

    import ml_dtypes

    xf = np.ascontiguousarray(np.asarray(x, dtype=np.float32).reshape(M_TOTAL, K))
    # pack sign(W) into the fp8 on-chip layout: wp[p, (c,b), o] = sign(W)[o, i],
    # i = 256c + 2p + b  (weight repacking, done once on host)
    sT = np.sign(np.asarray(W, dtype=np.float32)).T.astype(ml_dtypes.float8_e4m3)
    wp = np.ascontiguousarray(
        sT.reshape(4, P, 2, N).transpose(1, 0, 2, 3).reshape(P, 8 * N)
    )
    in_maps = [
        {"x": xf[i * M_PER_CORE : (i + 1) * M_PER_CORE], "W": wp}
        for i in range(N_CORES)
    ]
    nc = _get_nc(M_PER_CORE)
    res = bass_utils.run_bass_kernel_spmd(
        nc, in_maps, core_ids=list(range(N_CORES)), trace=_trace
    )
    out = np.concatenate([r["out"] for r in res.results], axis=0)
    out = out.reshape(4, 4096, N).astype(np.float32)
    if _trace:
        kernel.last_results = res
    return out
